# revision 45
# baseline (speedup 1.0000x reference)
"""Trainium2 Bass kernel for an 8-head AttentionBlock (B=4, C=512, H=W=32).

Sharding: 8 cores; core c handles batch b=c//2, query half hf=c%2 (512 query
rows), all 8 heads. The k/v projection is computed for the full batch on both
cores of a pair so no cross-core communication is needed.

v2 changes vs the 113us baseline (measured ~79-84us):
- PE warmup: 24 dummy matmuls on garbage data cover the initial DMA window so
  the TensorEngine reaches its max p-state before the first real matmul (the
  p-state ramp costs ~1.5-3x on the first ~3us after any idle; every stall
  resets it, so the whole schedule is built to keep the PE stall-free).
- exp split across engines: softmax exp (64 chunks of [128,512]) would be
  27us+ on Scalar alone. Scalar keeps 3/4 of the chunks (exact, AF.Exp); the
  DVE computes 1/4 with a Schraudolph-style approximation (score*A+B written
  as int16 == the bf16 bit pattern of exp(score*scale); rel err ~3e-3 on the
  final output, budget 2e-2).
- fine-grained weave: scores chunks are produced ~2.6x faster than exp drains
  them and PSUM holds only a 5-tile lead, so filler PE work (v-projection
  chunks, the hp=2/3 q/k chains, later the attn@v halves) is interleaved
  between scores tiles to keep the PE busy while exp catches up.
- uniform [128,512] single-bank psums: psc pool 5 banks rotating + 3
  dedicated attn@v banks, so the normalize chain (recip+broadcast+mult) sits
  ~8us behind the matmuls that would reuse its bank.
- v bias folded into the output bias on the host (bo' = bo + Wo @ bv), the
  residual is read from the bf16 x tile already in SBUF -- no fp32 x DMA.
- epilogue: out-proj accumulates hd0-2 as one group and hd3 as a separate
  start=False group, so only the final matmul waits on the last head's
  normalize chain.
- NOT DoubleRow/fp8: halving matmul time below the exp drain rate makes the
  PE idle-stall, the DVFS clock drops, and everything runs at mid-speed --
  measured strictly slower despite 30% less PE work.
"""

import os
import sys
import types

sys.path.insert(0, "/opt/trn_rl_repo")


# Install the antenv.axon_hooks module if missing so NTFF profiling
# (trace=True / BASS_TRACE=1) works under axon.
def _install_axon_profile_hook():
    try:
        import antenv
    except ImportError:
        return
    if "antenv.axon_hooks" in sys.modules:
        return
    try:
        from antenv.axon_hooks import get_axon_ntff_profile_hook  # noqa: F401
        return  # real module exists
    except ImportError:
        pass
    mod = types.ModuleType("antenv.axon_hooks")
    mod._hook = None

    def set_axon_ntff_profile_hook(h):
        mod._hook = h

    def get_axon_ntff_profile_hook():
        return mod._hook

    mod.set_axon_ntff_profile_hook = set_axon_ntff_profile_hook
    mod.get_axon_ntff_profile_hook = get_axon_ntff_profile_hook
    sys.modules["antenv.axon_hooks"] = mod
    antenv.axon_hooks = mod
    try:
        from trn_agent_boot.trn_boot import _ntff_profile_via_ctypes

        so = "/opt/axon/libaxon_pjrt.so"
        if os.path.exists(so):
            set_axon_ntff_profile_hook(_ntff_profile_via_ctypes(so))
    except Exception:
        pass


_install_axon_profile_hook()

import numpy as np
from contextlib import ExitStack

import concourse.bass as bass  # noqa: F401
import concourse.bacc as bacc
import concourse.mybir as mybir
import concourse.tile as tile
from concourse.bass_utils import run_bass_kernel_spmd

F32 = mybir.dt.float32
F8 = mybir.dt.float8e4
BF16 = mybir.dt.bfloat16
I16 = mybir.dt.int16
NP_BF16 = mybir.dt.np(BF16)
AF = mybir.ActivationFunctionType
ALU = mybir.AluOpType

B, C, S = 4, 512, 1024  # batch, channels, spatial (H*W)
NH, DK = 8, 64
SCALE = DK ** -0.5
N_CORES = 8
SL = S // 2  # local query rows per core

N_WARMUP = 19  # dummy matmuls covering the input-DMA window (p-state ramp)

# Schraudolph exp -> bf16 bit pattern: bits16 = s*EXP_A + EXP_B, viewed bf16.
EXP_A = float(SCALE * 128.0 * np.log2(np.e))
EXP_B = 16250.75  # 127*128 - correction (calibrated in numpy; rel ~1e-3)


def _build():
    nc = bacc.Bacc("TRN2", target_bir_lowering=False, debug=False,
                   num_devices=N_CORES)

    xbf_d = nc.dram_tensor("xbf", [C, S], BF16, kind="ExternalInput").ap()
    # fp8 copies of x and [WqT | WkT] feed the q/k projections: same PE pace
    # as bf16 (1 cyc/row) but half the front-critical DMA bytes, so the
    # first real matmuls start ~2.5us earlier. v/scores/attn@v stay bf16.
    x8_d = nc.dram_tensor("x8", [C, S], F8, kind="ExternalInput").ap()
    qk8_d = nc.dram_tensor("qk8", [C, 1024], F8, kind="ExternalInput").ap()
    # vo_pack columns: [WvT | WoT]
    vo_d = nc.dram_tensor("vo_pack", [C, 1024], BF16, kind="ExternalInput").ap()
    # bpack columns: bq (4 chunks) | bk (4) | bo' (4); bo' = bo + Wo @ bv
    # (the v bias is folded into the output bias on the host, so v needs no
    # on-device bias add and the residual x is read from the bf16 xb tile).
    bp_d = nc.dram_tensor("bpack", [128, 12], F32, kind="ExternalInput").ap()
    out_d = nc.dram_tensor("out", [C, SL], F32, kind="ExternalOutput").ap()

    with tile.TileContext(nc) as tc, ExitStack() as ctx:
        cst = ctx.enter_context(tc.tile_pool(name="cst", bufs=1))
        ppool = ctx.enter_context(tc.tile_pool(name="pp", bufs=3))
        rpool = ctx.enter_context(tc.tile_pool(name="rp", bufs=2))
        opool = ctx.enter_context(tc.tile_pool(name="op", bufs=4))
        psc = ctx.enter_context(tc.tile_pool(name="psc", bufs=5, space="PSUM"))
        pres = ctx.enter_context(tc.tile_pool(name="pres", bufs=1, space="PSUM"))

        # ---- persistent SBUF tiles ----
        xb_sb = cst.tile([128, 4 * S], BF16, tag="xb", name="xb")  # bf16 x
        x8_sb = cst.tile([128, 4 * S], F8, tag="x8", name="x8")    # fp8 x
        qk_sb = cst.tile([128, 4 * 1024], F8, tag="wqk", name="wqk")
        vo_sb = cst.tile([128, 4 * 1024], BF16, tag="wvo", name="wvo")
        bp_sb = cst.tile([128, 12], F32, tag="bp", name="bp")
        garb = cst.tile([128, 512], BF16, tag="garb", name="garb")
        ones_sb = cst.tile([128, 8], F32, tag="ones", name="ones")
        qT = [cst.tile([128, SL], BF16, tag=f"qT{i}", name=f"qT{i}")
              for i in range(4)]
        kT = [cst.tile([128, S], BF16, tag=f"kT{i}", name=f"kT{i}")
              for i in range(4)]
        v_sb = [cst.tile([128, NH * 65], BF16, tag=f"v{i}", name=f"v{i}")
                for i in range(8)]
        resT = [cst.tile([128, SL], BF16, tag=f"resT{i}", name=f"resT{i}")
                for i in range(4)]

        def xb(kc):  # bf16 x chunk kc as [128, 1024]
            return xb_sb[:, kc * S:(kc + 1) * S]

        def x8(kc):  # fp8 x chunk kc as [128, 1024]
            return x8_sb[:, kc * S:(kc + 1) * S]

        def wq(kc):
            return qk_sb[:, kc * 1024:kc * 1024 + 512]

        def wk(kc):
            return qk_sb[:, kc * 1024 + 512:kc * 1024 + 1024]

        def wv(kc):
            return vo_sb[:, kc * 1024:kc * 1024 + 512]

        def wo(kc):
            return vo_sb[:, kc * 1024 + 512:kc * 1024 + 1024]

        # ---- input DMAs ----
        # garbage memset first on vector so PE warmup can start immediately.
        nc.vector.memset(garb[:], 1.0)
        nc.vector.memset(ones_sb[:], 1.0)
        # bf16 x: 2 chunks on sync, 2 on gpsimd; weights split sync/scalar
        # (qk first); small tensors + the fp32 residual x on gpsimd (last).
        for kc in range(2):
            nc.sync.dma_start(x8_sb[:, kc * S:(kc + 1) * S],
                              x8_d[kc * 128:(kc + 1) * 128, :])
        for kc in range(2, 4):
            nc.gpsimd.dma_start(x8_sb[:, kc * S:(kc + 1) * S],
                                x8_d[kc * 128:(kc + 1) * 128, :])
        for kc in range(4):
            nc.scalar.dma_start(qk_sb[:, kc * 1024:(kc + 1) * 1024],
                                qk8_d[kc * 128:(kc + 1) * 128, :])
        nc.gpsimd.dma_start(bp_sb[:], bp_d[:])

        def emit_late_dmas():
            # xbf/vo are not needed until the v fillers (~20us in), but DMA
            # queues run ahead, so without a dependency these 2MB would
            # compete with the critical x8/qk8 transfers for HBM bandwidth.
            # Tiny writes into each chunk's destination range create WAW
            # deps that hold the transfers until the q/k path is loaded.
            for kc in range(4):
                nc.vector.tensor_copy(xb_sb[0:1, kc * S:kc * S + 1],
                                      ones_sb[0:1, 0:1])
                nc.vector.tensor_copy(vo_sb[0:1, kc * 1024:kc * 1024 + 1],
                                      ones_sb[0:1, 0:1])
            for kc in range(2):
                nc.sync.dma_start(xb_sb[:, kc * S:(kc + 1) * S],
                                  xbf_d[kc * 128:(kc + 1) * 128, :])
            for kc in range(2, 4):
                nc.gpsimd.dma_start(xb_sb[:, kc * S:(kc + 1) * S],
                                    xbf_d[kc * 128:(kc + 1) * 128, :])
            for kc in range(4):
                nc.scalar.dma_start(vo_sb[:, kc * 1024:(kc + 1) * 1024],
                                    vo_d[kc * 128:(kc + 1) * 128, :])

        # ---- PE warmup: spin the TensorEngine while inputs stream in ----
        ps_w = psc.tile([128, 512], F32, tag="sc", name="sc")
        for _ in range(N_WARMUP):
            nc.tensor.matmul(ps_w[:], garb[:, 0:128], garb[:],
                             start=True, stop=True)

        def emit_q(hp):
            # qT[hp] = Wq[hp-chunk] @ xs_local^T + bq  (features on partitions)
            ps = psc.tile([128, 512], F32, tag="sc", name="sc")
            for kc in range(4):
                nc.tensor.matmul(
                    ps[:],
                    wq(kc)[:, hp * 128:(hp + 1) * 128],
                    x8(kc)[:, 0:SL],
                    start=(kc == 0), stop=(kc == 3),
                )
            nc.vector.tensor_scalar_add(qT[hp][:], ps[:],
                                        bp_sb[:, hp:hp + 1])

        def emit_k(hp, ns):
            # kT[hp] half ns of the full batch sequence (512 keys)
            ps = psc.tile([128, 512], F32, tag="sc", name="sc")
            for kc in range(4):
                nc.tensor.matmul(
                    ps[:],
                    wk(kc)[:, hp * 128:(hp + 1) * 128],
                    x8(kc)[:, ns * 512:(ns + 1) * 512],
                    start=(kc == 0), stop=(kc == 3),
                )
            nc.vector.tensor_scalar_add(kT[hp][:, ns * 512:(ns + 1) * 512],
                                        ps[:], bp_sb[:, 4 + hp:5 + hp])

        def emit_qkT(hp):
            emit_q(hp)
            emit_k(hp, 0)
            emit_k(hp, 1)

        def emit_v(rc):
            # v in natural layout [rows, feat] with a ones column per head:
            # v_sb[rc] cols: head h occupies [h*65, h*65+64), col h*65+64 == 1
            # v psums live in the pres pool (idle until attn@v) so the
            # interleaved scores tiles keep the full psc rotation depth.
            ps = psc.tile([128, 512], F32, tag="sc", name="sc")
            for kc in range(4):
                nc.tensor.matmul(
                    ps[:],
                    xb(kc)[:, rc * 128:(rc + 1) * 128],
                    wv(kc),
                    start=(kc == 0), stop=(kc == 3),
                )
            vg = v_sb[rc][:].rearrange("p (h e) -> p h e", e=65)
            nc.gpsimd.tensor_copy(vg[:, :, 64], ones_sb[:])
            nc.vector.tensor_copy(vg[:, :, 0:64],
                                  ps[:].rearrange("p (h e) -> p h e", e=64))

        def sc_tile(hp, P, idx):
            # one scoresT tile for (half, hi): two [128, 512] psum chunks,
            # exp per chunk split Scalar (exact) / DVE (Schraudolph bits).
            half, hi = idx // 2, idx % 2
            base = hi * 64
            for j in range(2):
                kc = half * 2 + j
                ps = psc.tile([128, 512], F32, tag="sc", name="sc")
                nc.tensor.matmul(
                    ps[:],
                    kT[hp][base:base + 64, kc * 128:(kc + 1) * 128],
                    qT[hp][base:base + 64, :],
                    start=True, stop=True,
                )
                dst = P[hi][:, half * 1024 + j * 512:half * 1024 + j * 512 + 512]
                if idx in (1, 5):  # 2 of 8 tiles approximated on DVE
                    nc.vector.tensor_scalar(
                        dst.bitcast(I16), ps[:], EXP_A, EXP_B,
                        op0=ALU.mult, op1=ALU.add,
                    )
                else:
                    nc.scalar.activation(dst, ps[:], AF.Exp, scale=float(SCALE))

        _av_slot = [0]

        def emit_attnv_half(hp, P, hi):
            # attn @ v_ext (ones column -> row 64 = softmax denominator).
            # av psums rotate over 3 dedicated banks so the WAR on the
            # normalize chain sits ~8us behind the matmuls.
            h = hp * 2 + hi
            slot = _av_slot[0] % 3
            _av_slot[0] += 1
            pr = pres.tile([128, 512], F32, tag=f"r{slot}", name=f"r{slot}")
            for kc in range(8):
                nc.tensor.matmul(
                    pr[0:65, :],
                    v_sb[kc][:, h * 65:h * 65 + 65],
                    P[hi][:, kc * SL:(kc + 1) * SL],
                    start=(kc == 0), stop=(kc == 7),
                )
            # stage the denominator row to partition 0 (custom DVE ops
            # misread at base_partition != 0), invert, broadcast (GpSimd),
            # then normalize rows 0:63 straight out of PSUM (DVE mult).
            dn_t = rpool.tile([1, 512], F32, tag=f"dn{hi}", name=f"dn{hi}")
            nc.scalar.activation(dn_t[:], pr[64:65, :], AF.Copy)
            rc_t = rpool.tile([1, 512], F32, tag=f"rc{hi}", name=f"rc{hi}")
            nc.vector.reciprocal_approx_fast(rc_t[:], dn_t[:])
            rb_t = rpool.tile([64, 512], F32, tag=f"rb{hi}", name=f"rb{hi}")
            nc.gpsimd.partition_broadcast(rb_t[:], rc_t[0:1, :])
            nc.vector.tensor_tensor(
                resT[hp][hi * 64:(hi + 1) * 64, :],
                pr[0:64, :], rb_t[:], op=ALU.mult,
            )

        # ---- emission schedule ----
        def new_P():
            return [ppool.tile([128, 8 * SL], BF16, tag=f"P{i}", name=f"P{i}")
                    for i in range(2)]

        # Fine-grained interleave: scores tiles are produced ~2.6x faster
        # than the exp ops drain them (psc holds only 3 tiles of lead), so
        # filler PE work -- v-projection chunks and the hp=2/3 q/k chains
        # early, attn@v halves late -- is woven between scores tiles to keep
        # the PE busy while exp catches up.
        Ps = {}
        emit_qkT(0)
        emit_late_dmas()
        emit_qkT(1)
        fillers = [lambda: emit_q(2), lambda: emit_k(2, 0),
                   lambda: emit_k(2, 1), lambda: emit_q(3),
                   lambda: emit_k(3, 0), lambda: emit_k(3, 1)]
        fillers += [lambda rc=rc: emit_v(rc) for rc in range(8)]
        fit = iter(fillers)
        Ps[0] = new_P()
        Ps[1] = new_P()
        n_fill = [2, 2, 2, 2, 2, 2, 1, 1]
        for pair in range(8):
            hp = pair // 4
            sc_tile(hp, Ps[hp], 2 * (pair % 4))
            sc_tile(hp, Ps[hp], 2 * (pair % 4) + 1)
            for _ in range(n_fill[pair]):
                next(fit)()
        Ps[2] = new_P()
        emit_attnv_half(0, Ps[0], 0)
        for i in range(4):
            sc_tile(2, Ps[2], i)
        emit_attnv_half(0, Ps[0], 1)
        for i in range(4, 8):
            sc_tile(2, Ps[2], i)
        Ps.pop(0)
        Ps[3] = new_P()
        emit_attnv_half(1, Ps[1], 0)
        for i in range(4):
            sc_tile(3, Ps[3], i)
        emit_attnv_half(1, Ps[1], 1)
        for i in range(4, 8):
            sc_tile(3, Ps[3], i)
        Ps.pop(1)
        emit_attnv_half(2, Ps[2], 0)
        emit_attnv_half(2, Ps[2], 1)
        emit_attnv_half(3, Ps[3], 1)
        emit_attnv_half(3, Ps[3], 0)

        # ---- output projection + residual (fused epilogue) ----
        # hd 0-2 run as their own accumulation group so they overlap the
        # last attn@v normalize chain; only the hd=3 matmul (a separate
        # start=False accumulation) waits on resT[3].
        out_ps = []
        for cc in range(4):
            ps = psc.tile([128, 512], F32, tag="sc", name="sc")
            out_ps.append(ps)
            for hd in range(3):
                nc.tensor.matmul(
                    ps[:],
                    wo(hd)[:, cc * 128:(cc + 1) * 128],
                    resT[hd][:],
                    start=(hd == 0), stop=(hd == 2),
                )
        for cc in range(4):
            ps = out_ps[cc]
            nc.tensor.matmul(
                ps[:],
                wo(3)[:, cc * 128:(cc + 1) * 128],
                resT[3][:],
                start=False, stop=True, skip_group_check=True,
            )
            ot = opool.tile([128, SL], F32, tag="ob", name="ob")
            nc.vector.scalar_tensor_tensor(
                ot[:], ps[:], bp_sb[:, 8 + cc:9 + cc],
                xb(cc)[:, 0:SL],
                op0=ALU.add, op1=ALU.add,
            )
            eng = nc.sync if cc % 2 == 0 else nc.scalar
            eng.dma_start(out_d[cc * 128:(cc + 1) * 128, :], ot[:])

    nc.compile()
    return nc


_NC_CACHE = None


def _get_nc():
    global _NC_CACHE
    if _NC_CACHE is None:
        _NC_CACHE = _build()
    return _NC_CACHE


def _prep_inputs(x, Wp, bp, Wo, bo):
    """Host-side reshape/reorder of weights; returns per-core input maps."""
    x = np.ascontiguousarray(x, dtype=np.float32)
    Wp = np.asarray(Wp, dtype=np.float32)
    bp = np.asarray(bp, dtype=np.float32)
    Wo = np.asarray(Wo, dtype=np.float32)
    bo = np.asarray(bo, dtype=np.float32)

    # Wp rows per head h: [h*192, h*192+64) = q, +64..128 = k, +128..192 = v
    Wp3 = Wp.reshape(NH, 3, DK, C)
    Wq = Wp3[:, 0].reshape(NH * DK, C)
    Wk = Wp3[:, 1].reshape(NH * DK, C)
    Wv = Wp3[:, 2].reshape(NH * DK, C)
    bp3 = bp.reshape(NH, 3, DK)
    bq = bp3[:, 0].reshape(-1)
    bk = bp3[:, 1].reshape(-1)
    bv = bp3[:, 2].reshape(-1)

    NP_F8 = mybir.dt.np(F8)
    qk_pack = np.concatenate([Wq.T, Wk.T], axis=1)
    vo_pack = np.concatenate([Wv.T, Wo.T], axis=1)
    bo_p = bo + Wo @ bv  # fold the v bias into the output bias
    bpack = np.concatenate(
        [bq.reshape(4, 128).T, bk.reshape(4, 128).T, bo_p.reshape(4, 128).T],
        axis=1)

    shared = {
        "qk8": np.ascontiguousarray(qk_pack.astype(NP_F8)),
        "vo_pack": np.ascontiguousarray(vo_pack.astype(NP_BF16)),
        "bpack": np.ascontiguousarray(bpack.astype(np.float32)),
    }

    in_maps = []
    for c in range(N_CORES):
        b, hf = c // 2, c % 2
        xb = x[b].reshape(C, S)
        if hf == 0:
            xs = xb
        else:
            xs = np.concatenate([xb[:, SL:], xb[:, :SL]], axis=1)
        m = dict(shared)
        m["xbf"] = np.ascontiguousarray(xs.astype(NP_BF16))
        m["x8"] = np.ascontiguousarray(xs.astype(NP_F8))
        in_maps.append(m)
    return in_maps


def kernel(x, Wp, bp, Wo, bo):
    nc = _get_nc()
    in_maps = _prep_inputs(x, Wp, bp, Wo, bo)
    res = run_bass_kernel_spmd(nc, in_maps, list(range(N_CORES)))
    out = np.empty((B, C, S), dtype=np.float32)
    for c in range(N_CORES):
        b, hf = c // 2, c % 2
        out[b][:, hf * SL:(hf + 1) * SL] = res.results[c]["out"]
    H = int(np.sqrt(S))
    return out.reshape(B, C, H, H)
